# revision 15
# baseline (speedup 1.0000x reference)
"""Trainium2 Bass kernel for a Bahdanau attention decoder step (B=1).

Shapes (hardcoded): H=1024, V=50257, L=2048, B=1, 8 NeuronCores.

Sharding:
  - vocab dim padded to 51200 and sharded 6400/core for the output
    projection (out_W staged pre-transposed per core: [2H, 6400]).
  - embedding column-sharded [V, 128] per core; the looked-up row slice is
    AllGather'ed into the full embedded vector `we`.
  - GRU sharded by gate-slice: core k computes rows k*128:(k+1)*128 of each
    of the r/z/n gates and its h_new slice; h_new is AllGather'ed.
  - attention replicated (encoder_outputs staged transposed + attn_W[:, H:]).

Collectives: AllGather(we) at t~0 (overlapped), AllGather(h_new),
AllGather(logsumexp stats). log_softmax = logits - (M + log sum_j S_j e^{m_j-M}).

Perf notes: DMA spread across the sync+scalar HWDGE queues in ~1MB tiles; the
vocab projection and attention-score matmuls stream as float32r (same fp32
bytes, PE streams 1 row/cycle vs 4 for strict fp32); context and GRU math stay
strict fp32; softmax and log-softmax stats run on partition 0 (no transposes).
"""
import sys, os, ctypes, contextlib

if "/opt/trn_rl_repo" not in sys.path:
    sys.path.insert(0, "/opt/trn_rl_repo")

import numpy as np

H = 1024
V = 50257
L = 2048
NC = 8
VP = 51200          # padded vocab
VS = VP // NC       # 6400 per core
HC = H // 128       # 8 hidden chunks
DEBUG = bool(int(os.environ.get("NN_KERNEL_DEBUG", "0")))

_CACHE = {}


def _build():
    import concourse.bass as bass
    import concourse.bacc as bacc
    import concourse.mybir as mybir
    import concourse.tile as tile

    f32 = mybir.dt.float32
    f32r = mybir.dt.float32r
    i32 = mybir.dt.int32
    AF = mybir.ActivationFunctionType
    ALU = mybir.AluOpType
    AX = mybir.AxisListType

    nc = bacc.Bacc("TRN2", target_bir_lowering=False, debug=False, num_devices=NC)

    # ---- inputs (per-core shards staged by host) ----
    idx2 = nc.dram_tensor("idx2", [2, 1], i32, kind="ExternalInput")
    emb_cs = nc.dram_tensor("emb_cs", [V, 128], f32, kind="ExternalInput")
    encT = nc.dram_tensor("encT", [H, L], f32, kind="ExternalInput")
    w2T = nc.dram_tensor("w2T", [H, H], f32, kind="ExternalInput")
    vvec = nc.dram_tensor("vvec", [1, H], f32, kind="ExternalInput")
    wih = nc.dram_tensor("wih", [384, 2 * H], f32, kind="ExternalInput")
    whh = nc.dram_tensor("whh", [384, H], f32, kind="ExternalInput")
    bih = nc.dram_tensor("bih", [3, 128], f32, kind="ExternalInput")
    bhh = nc.dram_tensor("bhh", [3, 128], f32, kind="ExternalInput")
    hfull = nc.dram_tensor("hfull", [1, H], f32, kind="ExternalInput")
    hsh = nc.dram_tensor("hsh", [128, 1], f32, kind="ExternalInput")
    wot = nc.dram_tensor("wot", [2 * H, VS], f32, kind="ExternalInput")
    outb = nc.dram_tensor("outb", [1, VS], f32, kind="ExternalInput")

    # ---- outputs ----
    o_logp = nc.dram_tensor("o_logp", [1, VS], f32, kind="ExternalOutput")
    o_hnew = nc.dram_tensor("o_hnew", [128, 1], f32, kind="ExternalOutput")
    o_attnw = nc.dram_tensor("o_attnw", [1, L], f32, kind="ExternalOutput")
    if DEBUG:
        d_scores = nc.dram_tensor("d_scores", [1, L], f32, kind="ExternalOutput")
        d_ctx = nc.dram_tensor("d_ctx", [1, H], f32, kind="ExternalOutput")
        d_lg = nc.dram_tensor("d_lg", [1, VS], f32, kind="ExternalOutput")

    RG = [list(range(NC))]

    with tile.TileContext(nc) as tc:
        with tc.tile_pool(name="dram", bufs=1, space="DRAM") as dram, \
             tc.tile_pool(name="cst", bufs=1) as cst, \
             tc.tile_pool(name="big_p", bufs=12) as big_p, \
             tc.tile_pool(name="g_p", bufs=1) as g_p, \
             tc.tile_pool(name="scr", bufs=1) as scr_p, \
             tc.tile_pool(name="row_p", bufs=1) as row_p, \
             tc.tile_pool(name="rowc_p", bufs=2) as rowc_p, \
             tc.tile_pool(name="ps_acc", bufs=4, space="PSUM") as ps_acc:

            dmai = [0]
            dma_engs = [nc.sync, nc.scalar]

            def dma(out_ap, in_ap):
                e = dma_engs[dmai[0] % 2]
                dmai[0] += 1
                e.dma_start(out_ap, in_ap)

            # ---- 1. embedding gather + AllGather(we) (fires immediately) ----
            idx_sb = cst.tile([2, 1], i32)
            nc.sync.dma_start(idx_sb[:], idx2[:])
            we_g = cst.tile([2, 128], f32)
            nc.gpsimd.indirect_dma_start(
                out=we_g[:], out_offset=None, in_=emb_cs[:],
                in_offset=bass.IndirectOffsetOnAxis(ap=idx_sb[:, :1], axis=0))
            cwe_in = dram.tile([1, 128], f32)
            cwe_out = dram.tile([1, H], f32, addr_space="Shared")
            nc.sync.dma_start(cwe_in[:], we_g[0:1, :])
            nc.gpsimd.collective_compute(
                "AllGather", mybir.AluOpType.bypass, replica_groups=RG,
                ins=[cwe_in[:].opt()], outs=[cwe_out[:].opt()])

            # ---- 2. u2 = v @ W2 (DVE over W2T tiles, direct chunk layout) ----
            v_rep = cst.tile([128, H], f32)
            nc.sync.dma_start(v_rep[:], vvec[0:1, :].to_broadcast([128, H]))
            u2c_f = cst.tile([128, HC], f32)
            for mp in range(4):
                w2_t = big_p.tile([128, 2 * H], f32, tag="big", name=f"w2_{mp}")
                dma(w2_t[:].rearrange("p (b h) -> p b h", b=2),
                    w2T[mp * 256:(mp + 1) * 256, :]
                    .rearrange("(b p) h -> p b h", p=128))
                for b in range(2):
                    mc = 2 * mp + b
                    sc = scr_p.tile([128, 2 * H], f32, tag="scr", name=f"uscr_{mc}")
                    nc.vector.tensor_tensor(out=sc[:, 0:H],
                                            in0=w2_t[:, b * H:(b + 1) * H],
                                            in1=v_rep[:], op=ALU.mult)
                    nc.vector.tensor_reduce(u2c_f[:, mc:mc + 1], sc[:, 0:H],
                                            axis=AX.X, op=ALU.add)
            u2_c = cst.tile([128, HC], f32r)
            nc.scalar.copy(u2_c[:], u2c_f[:])

            # ---- 3. scores = enc @ u2 (PE, f32r over resident encT tiles) ----
            enc_t = []
            ps_s = [ps_acc.tile([1, 1024], f32, tag="acc", name=f"ps_s{i}",
                                space="PSUM") for i in range(2)]
            for hc in range(HC):
                et = big_p.tile([128, L], f32r, tag="big", name=f"enc_{hc}")
                dma(et[:], encT[hc * 128:(hc + 1) * 128, :].bitcast(f32r))
                enc_t.append(et)
                for j in range(0, L, 512):
                    nc.tensor.matmul(ps_s[j // 1024][0:1, j % 1024:j % 1024 + 512],
                                     lhsT=u2_c[:, hc:hc + 1],
                                     rhs=et[:, j:j + 512],
                                     start=(hc == 0), stop=(hc == HC - 1),
                                     skip_group_check=True)
            scores = cst.tile([1, L], f32)
            nc.vector.tensor_copy(scores[:, 0:1024], ps_s[0][0:1, :])
            nc.vector.tensor_copy(scores[:, 1024:2048], ps_s[1][0:1, :])
            if DEBUG:
                nc.sync.dma_start(d_scores[:], scores[:])

            # ---- 4. softmax(scores) on partition 0 ----
            sm = cst.tile([1, 1], f32)
            nc.vector.tensor_reduce(sm[:], scores[:], axis=AX.X, op=ALU.max)
            nsm = cst.tile([1, 1], f32)
            nc.vector.tensor_scalar_mul(nsm[:], sm[:], -1.0)
            exps = cst.tile([1, L], f32)
            zs = cst.tile([1, 1], f32)
            nc.scalar.activation(exps[:], scores[:], AF.Exp,
                                 bias=nsm[:], scale=1.0, accum_out=zs[:])
            exps_dram = dram.tile([1, L], f32)
            nc.sync.dma_start(exps_dram[:], exps[:])
            exps_rep = cst.tile([128, L], f32)
            nc.sync.dma_start(exps_rep[:], exps_dram[:].to_broadcast([128, L]))
            rz = cst.tile([1, 1], f32)
            nc.vector.reciprocal(rz[:], zs[:])
            attw_row = cst.tile([1, L], f32)
            nc.scalar.mul(attw_row[:], exps[:], rz[:])
            nc.sync.dma_start(o_attnw[:], attw_row[:])
            rz_dram = dram.tile([1, 1], f32)
            nc.sync.dma_start(rz_dram[:], rz[:])
            rz_bc = cst.tile([128, 1], f32)
            nc.sync.dma_start(rz_bc[:], rz_dram[:].to_broadcast([128, 1]))

            # ---- 5. context = attw @ enc (DVE, fp32 exact) ----
            ctx_u = cst.tile([128, HC], f32)
            for hc in range(HC):
                sc = scr_p.tile([128, 2 * H], f32, tag="scr", name=f"cscr_{hc}")
                nc.vector.tensor_tensor(out=sc[:], in0=enc_t[hc][:].bitcast(f32),
                                        in1=exps_rep[:], op=ALU.mult)
                nc.vector.tensor_reduce(ctx_u[:, hc:hc + 1], sc[:], axis=AX.X,
                                        op=ALU.add)
            ctx_c = cst.tile([128, HC], f32)
            nc.vector.tensor_scalar(out=ctx_c[:], in0=ctx_u[:], scalar1=rz_bc[:, 0:1],
                                    scalar2=None, op0=ALU.mult)
            ctx_dram = dram.tile([1, H], f32)
            nc.sync.dma_start(ctx_dram[0:1, :].rearrange("a (c p) -> (a p) c", p=128),
                              ctx_c[:])
            if DEBUG:
                nc.sync.dma_start(d_ctx[:], ctx_dram[:])

            # ---- 6. GRU slice-k (DVE, fp32) ----
            h_rep = cst.tile([128, H], f32)
            nc.sync.dma_start(h_rep[:], hfull[0:1, :].to_broadcast([128, H]))
            ghcol = cst.tile([128, 3], f32)
            wt01 = g_p.tile([128, 2 * H], f32, tag="gw", name="whh01")
            dma(wt01[:].rearrange("p (b h) -> p b h", b=2),
                whh[0:256, :].rearrange("(b p) h -> p b h", p=128))
            for g in range(2):
                sc = scr_p.tile([128, 2 * H], f32, tag="scr", name=f"ghscr_{g}")
                nc.vector.tensor_tensor(out=sc[:, 0:H], in0=wt01[:, g * H:(g + 1) * H],
                                        in1=h_rep[:], op=ALU.mult)
                nc.vector.tensor_reduce(ghcol[:, g:g + 1], sc[:, 0:H], axis=AX.X,
                                        op=ALU.add)
            wt2 = g_p.tile([128, 2 * H], f32, tag="gw", name="whh2")
            dma(wt2[:, 0:H], whh[256:384, :])
            sc = scr_p.tile([128, 2 * H], f32, tag="scr", name="ghscr_2")
            nc.vector.tensor_tensor(out=sc[:, 0:H], in0=wt2[:, 0:H], in1=h_rep[:],
                                    op=ALU.mult)
            nc.vector.tensor_reduce(ghcol[:, 2:3], sc[:, 0:H], axis=AX.X, op=ALU.add)

            x_rep = cst.tile([128, 2 * H], f32)
            nc.sync.dma_start(x_rep[:, 0:H], cwe_out[:].to_broadcast([128, H]))
            nc.sync.dma_start(x_rep[:, H:2 * H], ctx_dram[:].to_broadcast([128, H]))
            gicol = cst.tile([128, 3], f32)
            for g in range(3):
                wt = g_p.tile([128, 2 * H], f32, tag="gw", name=f"wih_{g}")
                dma(wt[:], wih[g * 128:(g + 1) * 128, :])
                sc = scr_p.tile([128, 2 * H], f32, tag="scr", name=f"giscr_{g}")
                nc.vector.tensor_tensor(out=sc[:], in0=wt[:], in1=x_rep[:],
                                        op=ALU.mult)
                nc.vector.tensor_reduce(gicol[:, g:g + 1], sc[:], axis=AX.X,
                                        op=ALU.add)

            bih_sb = cst.tile([128, 3], f32)
            nc.sync.dma_start(bih_sb[:], bih[:].rearrange("g p -> p g"))
            bhh_sb = cst.tile([128, 3], f32)
            nc.sync.dma_start(bhh_sb[:], bhh[:].rearrange("g p -> p g"))
            gib = cst.tile([128, 3], f32)
            nc.vector.tensor_tensor(out=gib[:], in0=gicol[:], in1=bih_sb[:], op=ALU.add)
            ghb = cst.tile([128, 3], f32)
            nc.vector.tensor_tensor(out=ghb[:], in0=ghcol[:], in1=bhh_sb[:], op=ALU.add)

            rzpre = cst.tile([128, 2], f32)
            nc.vector.tensor_tensor(out=rzpre[:], in0=gib[:, 0:2], in1=ghb[:, 0:2],
                                    op=ALU.add)
            rzg = cst.tile([128, 2], f32)
            nc.scalar.activation(rzg[:], rzpre[:], AF.Sigmoid)
            npre = cst.tile([128, 1], f32)
            nc.vector.tensor_tensor(out=npre[:], in0=rzg[:, 0:1], in1=ghb[:, 2:3],
                                    op=ALU.mult)
            nc.vector.tensor_tensor(out=npre[:], in0=npre[:], in1=gib[:, 2:3],
                                    op=ALU.add)
            ngate = cst.tile([128, 1], f32)
            nc.scalar.activation(ngate[:], npre[:], AF.Tanh)
            hsh_sb = cst.tile([128, 1], f32)
            nc.sync.dma_start(hsh_sb[:], hsh[:])
            hmn = cst.tile([128, 1], f32)
            nc.vector.tensor_tensor(out=hmn[:], in0=hsh_sb[:], in1=ngate[:],
                                    op=ALU.subtract)
            zh = cst.tile([128, 1], f32)
            nc.vector.tensor_tensor(out=zh[:], in0=rzg[:, 1:2], in1=hmn[:],
                                    op=ALU.mult)
            hnew = cst.tile([128, 1], f32)
            nc.vector.tensor_tensor(out=hnew[:], in0=ngate[:], in1=zh[:], op=ALU.add)
            nc.sync.dma_start(o_hnew[:], hnew[:])

            # ---- 7. AllGather(h_new) ----
            c4_in = dram.tile([1, 128], f32)
            nc.sync.dma_start(c4_in[0:1, :].rearrange("a b -> b a"), hnew[:])
            c4_out = dram.tile([1, H], f32, addr_space="Shared")
            nc.gpsimd.collective_compute(
                "AllGather", mybir.AluOpType.bypass, replica_groups=RG,
                ins=[c4_in[:].opt()], outs=[c4_out[:].opt()])

            # ---- 8. logits = [h_new; ctx] @ out_W_shard.T (PE, f32r) ----
            x2c = cst.tile([128, 2 * HC], f32r)
            nc.sync.dma_start(x2c[:, 0:HC],
                              c4_out[0:1, :].rearrange("a (c p) -> (a p) c", p=128)
                              .bitcast(f32r))
            nc.sync.dma_start(x2c[:, HC:2 * HC],
                              ctx_dram[0:1, :].rearrange("a (c p) -> (a p) c", p=128)
                              .bitcast(f32r))

            lg_row = row_p.tile([1, VS], f32, tag="lg", name="lg_row")
            nck = (VS + 1023) // 1024
            mcs = cst.tile([1, nck], f32)
            nmcs = cst.tile([1, nck], f32)
            scs = cst.tile([1, nck], f32)
            VBP = [(0, 2048), (2048, 2048), (4096, 2048), (6144, 256)]
            halves = [("c", list(range(HC, 2 * HC))), ("h", list(range(HC)))]
            for half, ks in halves:
                for off, wid in VBP:
                    nvb = (wid + 1023) // 1024
                    pss = [ps_acc.tile([1, 1024], f32, tag="acc",
                                       name=f"psl_{half}_{off}_{vb}", space="PSUM")
                           for vb in range(nvb)]
                    for i, k in enumerate(ks):
                        wo_fp = big_p.tile([128, 2048], f32r, tag="big",
                                           name=f"wofp_{half}_{off}_{k}")
                        dma(wo_fp[:, 0:wid],
                            wot[k * 128:(k + 1) * 128, off:off + wid].bitcast(f32r))
                        for vb in range(nvb):
                            w0 = vb * 1024
                            vw = min(1024, wid - w0)
                            for j in range(0, vw, 512):
                                nj = min(512, vw - j)
                                nc.tensor.matmul(
                                    pss[vb][0:1, j:j + nj],
                                    lhsT=x2c[:, k:k + 1],
                                    rhs=wo_fp[:, w0 + j:w0 + j + nj],
                                    start=(i == 0), stop=(i == HC - 1),
                                    skip_group_check=True)
                    for vb in range(nvb):
                        w0 = vb * 1024
                        vw = min(1024, wid - w0)
                        seg = lg_row[0:1, off + w0:off + w0 + vw]
                        if half == "c":
                            nc.vector.tensor_copy(seg, pss[vb][0:1, 0:vw])
                        else:
                            ci = (off + w0) // 1024
                            nc.vector.tensor_tensor(out=seg, in0=seg,
                                                    in1=pss[vb][0:1, 0:vw],
                                                    op=ALU.add)
                            ob = rowc_p.tile([1, 1024], f32, tag="obc",
                                             name=f"ob_{off}_{vb}")
                            nc.sync.dma_start(ob[0:1, 0:vw],
                                              outb[0:1, off + w0:off + w0 + vw])
                            nc.vector.tensor_tensor(out=seg, in0=seg,
                                                    in1=ob[0:1, 0:vw], op=ALU.add)
                            nc.vector.tensor_reduce(mcs[:, ci:ci + 1], seg,
                                                    axis=AX.X, op=ALU.max)
                            nc.vector.tensor_scalar_mul(nmcs[:, ci:ci + 1],
                                                        mcs[:, ci:ci + 1], -1.0)
                            ex = rowc_p.tile([1, 1024], f32, tag="obc",
                                             name=f"ex_{off}_{vb}")
                            nc.scalar.activation(ex[0:1, 0:vw], seg, AF.Exp,
                                                 bias=nmcs[:, ci:ci + 1], scale=1.0,
                                                 accum_out=scs[:, ci:ci + 1])
            if DEBUG:
                nc.sync.dma_start(d_lg[:], lg_row[:])

            # ---- 9. merge chunk stats + AllGather ----
            lm = cst.tile([1, 1], f32)
            nc.vector.tensor_reduce(lm[:], mcs[:], axis=AX.X, op=ALU.max)
            nlm = cst.tile([1, 1], f32)
            nc.vector.tensor_scalar_mul(nlm[:], lm[:], -1.0)
            dch = cst.tile([1, nck], f32)
            nc.scalar.activation(dch[:], mcs[:], AF.Exp, bias=nlm[:], scale=1.0)
            tch = cst.tile([1, nck], f32)
            nc.vector.tensor_tensor(out=tch[:], in0=dch[:], in1=scs[:], op=ALU.mult)
            ls = cst.tile([1, 1], f32)
            nc.vector.tensor_reduce(ls[:], tch[:], axis=AX.X, op=ALU.add)
            st_sb = cst.tile([1, 2], f32)
            nc.vector.tensor_copy(st_sb[:, 0:1], lm[:])
            nc.vector.tensor_copy(st_sb[:, 1:2], ls[:])
            st_in = dram.tile([1, 64], f32)
            nc.sync.dma_start(st_in[0:1, 0:2], st_sb[:])
            st_out = dram.tile([1, 64 * NC], f32, addr_space="Shared")
            nc.gpsimd.collective_compute(
                "AllGather", mybir.AluOpType.bypass, replica_groups=RG,
                ins=[st_in[:].opt()], outs=[st_out[:].opt()])

            # ---- 10. global lse, final log-probs ----
            st_a = cst.tile([1, 64 * NC], f32)
            nc.sync.dma_start(st_a[:], st_out[:])
            st_v = st_a[:].rearrange("a (j r) -> a r j", r=64)   # [1, 64, 8]
            mvals = cst.tile([1, NC], f32)
            nc.vector.tensor_copy(mvals[:], st_v[:, 0:1, :])
            svals = cst.tile([1, NC], f32)
            nc.vector.tensor_copy(svals[:], st_v[:, 1:2, :])
            gM = cst.tile([1, 1], f32)
            nc.vector.tensor_reduce(gM[:], mvals[:], axis=AX.X, op=ALU.max)
            ngM = cst.tile([1, 1], f32)
            nc.vector.tensor_scalar_mul(ngM[:], gM[:], -1.0)
            dvals = cst.tile([1, NC], f32)
            nc.scalar.activation(dvals[:], mvals[:], AF.Exp, bias=ngM[:], scale=1.0)
            tvals = cst.tile([1, NC], f32)
            nc.vector.tensor_tensor(out=tvals[:], in0=dvals[:], in1=svals[:],
                                    op=ALU.mult)
            gZ = cst.tile([1, 1], f32)
            nc.vector.tensor_reduce(gZ[:], tvals[:], axis=AX.X, op=ALU.add)
            lnZ = cst.tile([1, 1], f32)
            nc.scalar.activation(lnZ[:], gZ[:], AF.Ln)
            lse = cst.tile([1, 1], f32)
            nc.vector.tensor_tensor(out=lse[:], in0=lnZ[:], in1=gM[:], op=ALU.add)
            nlse = cst.tile([1, 1], f32)
            nc.vector.tensor_scalar_mul(nlse[:], lse[:], -1.0)
            nc.scalar.activation(lg_row[:], lg_row[:], AF.Identity,
                                 bias=nlse[:], scale=1.0)
            nc.sync.dma_start(o_logp[:], lg_row[:])

    nc.compile()
    return nc


def _prep_in_maps(inputs):
    wi = np.asarray(inputs["word_input"]).astype(np.int64).reshape(-1)
    emb = np.asarray(inputs["emb"], dtype=np.float32)
    enc = np.asarray(inputs["encoder_outputs"], dtype=np.float32).reshape(L, H)
    attn_W = np.asarray(inputs["attn_W"], dtype=np.float32)
    v = np.asarray(inputs["v"], dtype=np.float32).reshape(1, H)
    W_ih = np.asarray(inputs["W_ih"], dtype=np.float32)
    W_hh = np.asarray(inputs["W_hh"], dtype=np.float32)
    b_ih = np.asarray(inputs["b_ih"], dtype=np.float32).reshape(-1)
    b_hh = np.asarray(inputs["b_hh"], dtype=np.float32).reshape(-1)
    h = np.asarray(inputs["last_hidden"], dtype=np.float32).reshape(1, H)
    out_W = np.asarray(inputs["out_W"], dtype=np.float32)
    out_b = np.asarray(inputs["out_b"], dtype=np.float32).reshape(-1)

    idx2 = np.full((2, 1), int(wi[0]), dtype=np.int32)
    w2T = np.ascontiguousarray(attn_W[:, H:2 * H].T)
    encT = np.ascontiguousarray(enc.T)

    Wpad = np.zeros((VP, 2 * H), dtype=np.float32)
    Wpad[:V] = out_W
    bpad = np.full((VP,), -1e30, dtype=np.float32)
    bpad[:V] = out_b

    in_maps = []
    for k in range(NC):
        rows = np.concatenate([np.arange(g * H + k * 128, g * H + (k + 1) * 128)
                               for g in range(3)])
        in_maps.append({
            "idx2": idx2,
            "emb_cs": np.ascontiguousarray(emb[:, k * 128:(k + 1) * 128]),
            "encT": encT,
            "w2T": w2T,
            "vvec": v,
            "wih": np.ascontiguousarray(W_ih[rows]),
            "whh": np.ascontiguousarray(W_hh[rows]),
            "bih": np.ascontiguousarray(b_ih[rows].reshape(3, 128)),
            "bhh": np.ascontiguousarray(b_hh[rows].reshape(3, 128)),
            "hfull": h,
            "hsh": np.ascontiguousarray(h[0, k * 128:(k + 1) * 128].reshape(128, 1)),
            "wot": np.ascontiguousarray(Wpad[k * VS:(k + 1) * VS].T),
            "outb": np.ascontiguousarray(bpad[k * VS:(k + 1) * VS].reshape(1, VS)),
        })
    return in_maps


@contextlib.contextmanager
def _maybe_profile():
    prof_dir = os.environ.get("NN_PROF_DIR")
    if not prof_dir:
        yield
        return
    import jax
    jax.devices()
    lib = ctypes.CDLL("/opt/axon/libaxon_pjrt.so")
    lib.axon_start_nrt_profile.argtypes = [ctypes.POINTER(ctypes.c_int64),
                                           ctypes.c_size_t]
    lib.axon_start_nrt_profile.restype = ctypes.c_int64
    lib.axon_stop_nrt_profile.argtypes = [ctypes.c_char_p]
    lib.axon_stop_nrt_profile.restype = ctypes.c_int64
    ids = (ctypes.c_int64 * 1)(0)
    rc = lib.axon_start_nrt_profile(ids, 1)
    if rc != 0:
        raise RuntimeError(f"axon_start_nrt_profile rc={rc}")
    try:
        yield
    finally:
        n = lib.axon_stop_nrt_profile(str(prof_dir).encode())
        print(f"profile: {n} file(s) written to {prof_dir}")


def kernel(**inputs):
    from concourse import bass_utils

    if "nc" not in _CACHE:
        _CACHE["nc"] = _build()
    nc = _CACHE["nc"]
    in_maps = _prep_in_maps(inputs)
    with _maybe_profile():
        res = bass_utils.run_bass_kernel_spmd(nc, in_maps, core_ids=list(range(NC)))

    if DEBUG:
        _CACHE["last_results"] = res.results

    logp = np.concatenate([res.results[k]["o_logp"].reshape(VS) for k in range(NC)])
    log_probs = logp[:V].reshape(1, V)
    h_new = np.concatenate([res.results[k]["o_hnew"].reshape(128)
                            for k in range(NC)]).reshape(1, 1, H)
    attn_w = res.results[0]["o_attnw"].reshape(L).reshape(1, 1, L)
    return log_probs, h_new, attn_w


# revision 16
# speedup vs baseline: 1.0313x; 1.0313x over previous
"""Trainium2 Bass kernel for a Bahdanau attention decoder step (B=1).

Shapes (hardcoded): H=1024, V=50257, L=2048, B=1, 8 NeuronCores.

Sharding:
  - vocab dim padded to 51200 and sharded 6400/core for the output
    projection (out_W staged pre-transposed per core: [2H, 6400]).
  - embedding column-sharded [V, 128] per core; the looked-up row slice is
    AllGather'ed into the full embedded vector `we`.
  - GRU sharded by gate-slice: core k computes rows k*128:(k+1)*128 of each
    of the r/z/n gates and its h_new slice; h_new is AllGather'ed.
  - attention replicated (encoder_outputs staged transposed + attn_W[:, H:]).

Collectives: AllGather(we) at t~0 (overlapped), AllGather(h_new),
AllGather(logsumexp stats). log_softmax = logits - (M + log sum_j S_j e^{m_j-M}).

Perf notes: DMA spread across the sync+scalar HWDGE queues in ~1MB tiles; the
vocab projection and attention-score matmuls stream as float32r (same fp32
bytes, PE streams 1 row/cycle vs 4 for strict fp32); context and GRU math stay
strict fp32; softmax and log-softmax stats run on partition 0 (no transposes).
"""
import sys, os, ctypes, contextlib

if "/opt/trn_rl_repo" not in sys.path:
    sys.path.insert(0, "/opt/trn_rl_repo")

import numpy as np

H = 1024
V = 50257
L = 2048
NC = 8
VP = 51200          # padded vocab
VS = VP // NC       # 6400 per core
HC = H // 128       # 8 hidden chunks
DEBUG = bool(int(os.environ.get("NN_KERNEL_DEBUG", "0")))

_CACHE = {}


def _build():
    import concourse.bass as bass
    import concourse.bacc as bacc
    import concourse.mybir as mybir
    import concourse.tile as tile

    f32 = mybir.dt.float32
    f32r = mybir.dt.float32r
    i32 = mybir.dt.int32
    AF = mybir.ActivationFunctionType
    ALU = mybir.AluOpType
    AX = mybir.AxisListType

    nc = bacc.Bacc("TRN2", target_bir_lowering=False, debug=False, num_devices=NC)

    # ---- inputs (per-core shards staged by host) ----
    idx2 = nc.dram_tensor("idx2", [2, 1], i32, kind="ExternalInput")
    emb_cs = nc.dram_tensor("emb_cs", [V, 128], f32, kind="ExternalInput")
    encT = nc.dram_tensor("encT", [H, L], f32, kind="ExternalInput")
    w2T = nc.dram_tensor("w2T", [H, H], f32, kind="ExternalInput")
    vvec = nc.dram_tensor("vvec", [1, H], f32, kind="ExternalInput")
    wih = nc.dram_tensor("wih", [384, 2 * H], f32, kind="ExternalInput")
    whh = nc.dram_tensor("whh", [384, H], f32, kind="ExternalInput")
    bih = nc.dram_tensor("bih", [3, 128], f32, kind="ExternalInput")
    bhh = nc.dram_tensor("bhh", [3, 128], f32, kind="ExternalInput")
    hfull = nc.dram_tensor("hfull", [1, H], f32, kind="ExternalInput")
    hsh = nc.dram_tensor("hsh", [128, 1], f32, kind="ExternalInput")
    wot = nc.dram_tensor("wot", [2 * H, VS], f32, kind="ExternalInput")
    outb = nc.dram_tensor("outb", [1, VS], f32, kind="ExternalInput")

    # ---- outputs ----
    o_logp = nc.dram_tensor("o_logp", [1, VS], f32, kind="ExternalOutput")
    o_hnew = nc.dram_tensor("o_hnew", [128, 1], f32, kind="ExternalOutput")
    o_attnw = nc.dram_tensor("o_attnw", [1, L], f32, kind="ExternalOutput")
    if DEBUG:
        d_scores = nc.dram_tensor("d_scores", [1, L], f32, kind="ExternalOutput")
        d_ctx = nc.dram_tensor("d_ctx", [1, H], f32, kind="ExternalOutput")
        d_lg = nc.dram_tensor("d_lg", [1, VS], f32, kind="ExternalOutput")

    RG = [list(range(NC))]

    with tile.TileContext(nc) as tc:
        with tc.tile_pool(name="dram", bufs=1, space="DRAM") as dram, \
             tc.tile_pool(name="cst", bufs=1) as cst, \
             tc.tile_pool(name="big_p", bufs=12) as big_p, \
             tc.tile_pool(name="g_p", bufs=1) as g_p, \
             tc.tile_pool(name="scr", bufs=1) as scr_p, \
             tc.tile_pool(name="row_p", bufs=1) as row_p, \
             tc.tile_pool(name="rowc_p", bufs=2) as rowc_p, \
             tc.tile_pool(name="ps_acc", bufs=4, space="PSUM") as ps_acc:

            dmai = [0]
            dma_engs = [nc.sync, nc.scalar]

            def dma(out_ap, in_ap):
                e = dma_engs[dmai[0] % 2]
                dmai[0] += 1
                e.dma_start(out_ap, in_ap)

            # ---- 1. embedding gather + AllGather(we) (fires immediately) ----
            idx_sb = cst.tile([2, 1], i32)
            nc.gpsimd.dma_start(idx_sb[:], idx2[:])
            we_g = cst.tile([2, 128], f32)
            nc.gpsimd.indirect_dma_start(
                out=we_g[:], out_offset=None, in_=emb_cs[:],
                in_offset=bass.IndirectOffsetOnAxis(ap=idx_sb[:, :1], axis=0))
            cwe_in = dram.tile([1, 128], f32)
            cwe_out = dram.tile([1, H], f32, addr_space="Shared")
            nc.gpsimd.dma_start(cwe_in[:], we_g[0:1, :])
            nc.gpsimd.collective_compute(
                "AllGather", mybir.AluOpType.bypass, replica_groups=RG,
                ins=[cwe_in[:].opt()], outs=[cwe_out[:].opt()])

            # ---- 2. u2 = v @ W2 (DVE over W2T tiles, direct chunk layout) ----
            v_rep = cst.tile([128, H], f32)
            nc.gpsimd.dma_start(v_rep[:], vvec[0:1, :].to_broadcast([128, H]))
            u2c_f = cst.tile([128, HC], f32)
            for mp in range(4):
                w2_t = big_p.tile([128, 2 * H], f32, tag="big", name=f"w2_{mp}")
                dma(w2_t[:].rearrange("p (b h) -> p b h", b=2),
                    w2T[mp * 256:(mp + 1) * 256, :]
                    .rearrange("(b p) h -> p b h", p=128))
                for b in range(2):
                    mc = 2 * mp + b
                    sc = scr_p.tile([128, 2 * H], f32, tag="scr", name=f"uscr_{mc}")
                    nc.vector.tensor_tensor(out=sc[:, 0:H],
                                            in0=w2_t[:, b * H:(b + 1) * H],
                                            in1=v_rep[:], op=ALU.mult)
                    nc.vector.tensor_reduce(u2c_f[:, mc:mc + 1], sc[:, 0:H],
                                            axis=AX.X, op=ALU.add)
            u2_c = cst.tile([128, HC], f32r)
            nc.scalar.copy(u2_c[:], u2c_f[:])

            # ---- 3. scores = enc @ u2 (PE, f32r over resident encT tiles) ----
            enc_t = []
            ps_s = [ps_acc.tile([1, 1024], f32, tag="acc", name=f"ps_s{i}",
                                space="PSUM") for i in range(2)]
            for hc in range(HC):
                et = big_p.tile([128, L], f32r, tag="big", name=f"enc_{hc}")
                dma(et[:], encT[hc * 128:(hc + 1) * 128, :].bitcast(f32r))
                enc_t.append(et)
                for j in range(0, L, 512):
                    nc.tensor.matmul(ps_s[j // 1024][0:1, j % 1024:j % 1024 + 512],
                                     lhsT=u2_c[:, hc:hc + 1],
                                     rhs=et[:, j:j + 512],
                                     start=(hc == 0), stop=(hc == HC - 1),
                                     skip_group_check=True)
            scores = cst.tile([1, L], f32)
            nc.vector.tensor_copy(scores[:, 0:1024], ps_s[0][0:1, :])
            nc.vector.tensor_copy(scores[:, 1024:2048], ps_s[1][0:1, :])
            if DEBUG:
                nc.sync.dma_start(d_scores[:], scores[:])

            # ---- 4. softmax(scores) on partition 0 ----
            sm = cst.tile([1, 1], f32)
            nc.vector.tensor_reduce(sm[:], scores[:], axis=AX.X, op=ALU.max)
            nsm = cst.tile([1, 1], f32)
            nc.vector.tensor_scalar_mul(nsm[:], sm[:], -1.0)
            exps = cst.tile([1, L], f32)
            zs = cst.tile([1, 1], f32)
            nc.scalar.activation(exps[:], scores[:], AF.Exp,
                                 bias=nsm[:], scale=1.0, accum_out=zs[:])
            exps_dram = dram.tile([1, L], f32)
            nc.gpsimd.dma_start(exps_dram[:], exps[:])
            exps_rep = cst.tile([128, L], f32)
            nc.gpsimd.dma_start(exps_rep[:], exps_dram[:].to_broadcast([128, L]))
            rz = cst.tile([1, 1], f32)
            nc.vector.reciprocal(rz[:], zs[:])
            attw_row = cst.tile([1, L], f32)
            nc.scalar.mul(attw_row[:], exps[:], rz[:])
            nc.sync.dma_start(o_attnw[:], attw_row[:])
            rz_dram = dram.tile([1, 1], f32)
            nc.gpsimd.dma_start(rz_dram[:], rz[:])
            rz_bc = cst.tile([128, 1], f32)
            nc.gpsimd.dma_start(rz_bc[:], rz_dram[:].to_broadcast([128, 1]))

            # ---- 5. context = attw @ enc (DVE, fp32 exact) ----
            ctx_u = cst.tile([128, HC], f32)
            for hc in range(HC):
                sc = scr_p.tile([128, 2 * H], f32, tag="scr", name=f"cscr_{hc}")
                nc.vector.tensor_tensor(out=sc[:], in0=enc_t[hc][:].bitcast(f32),
                                        in1=exps_rep[:], op=ALU.mult)
                nc.vector.tensor_reduce(ctx_u[:, hc:hc + 1], sc[:], axis=AX.X,
                                        op=ALU.add)
            ctx_c = cst.tile([128, HC], f32)
            nc.vector.tensor_scalar(out=ctx_c[:], in0=ctx_u[:], scalar1=rz_bc[:, 0:1],
                                    scalar2=None, op0=ALU.mult)
            ctx_dram = dram.tile([1, H], f32)
            nc.gpsimd.dma_start(ctx_dram[0:1, :].rearrange("a (c p) -> (a p) c", p=128),
                                ctx_c[:])
            if DEBUG:
                nc.sync.dma_start(d_ctx[:], ctx_dram[:])

            # ---- 6. GRU slice-k (DVE, fp32) ----
            h_rep = cst.tile([128, H], f32)
            nc.gpsimd.dma_start(h_rep[:], hfull[0:1, :].to_broadcast([128, H]))
            ghcol = cst.tile([128, 3], f32)
            wt01 = g_p.tile([128, 2 * H], f32, tag="gw", name="whh01")
            dma(wt01[:].rearrange("p (b h) -> p b h", b=2),
                whh[0:256, :].rearrange("(b p) h -> p b h", p=128))
            for g in range(2):
                sc = scr_p.tile([128, 2 * H], f32, tag="scr", name=f"ghscr_{g}")
                nc.vector.tensor_tensor(out=sc[:, 0:H], in0=wt01[:, g * H:(g + 1) * H],
                                        in1=h_rep[:], op=ALU.mult)
                nc.vector.tensor_reduce(ghcol[:, g:g + 1], sc[:, 0:H], axis=AX.X,
                                        op=ALU.add)
            wt2 = g_p.tile([128, 2 * H], f32, tag="gw", name="whh2")
            dma(wt2[:, 0:H], whh[256:384, :])
            sc = scr_p.tile([128, 2 * H], f32, tag="scr", name="ghscr_2")
            nc.vector.tensor_tensor(out=sc[:, 0:H], in0=wt2[:, 0:H], in1=h_rep[:],
                                    op=ALU.mult)
            nc.vector.tensor_reduce(ghcol[:, 2:3], sc[:, 0:H], axis=AX.X, op=ALU.add)

            x_rep = cst.tile([128, 2 * H], f32)
            nc.gpsimd.dma_start(x_rep[:, 0:H], cwe_out[:].to_broadcast([128, H]))
            nc.gpsimd.dma_start(x_rep[:, H:2 * H], ctx_dram[:].to_broadcast([128, H]))
            gicol = cst.tile([128, 3], f32)
            for g in range(3):
                wt = g_p.tile([128, 2 * H], f32, tag="gw", name=f"wih_{g}")
                dma(wt[:], wih[g * 128:(g + 1) * 128, :])
                sc = scr_p.tile([128, 2 * H], f32, tag="scr", name=f"giscr_{g}")
                nc.vector.tensor_tensor(out=sc[:], in0=wt[:], in1=x_rep[:],
                                        op=ALU.mult)
                nc.vector.tensor_reduce(gicol[:, g:g + 1], sc[:], axis=AX.X,
                                        op=ALU.add)

            bih_sb = cst.tile([128, 3], f32)
            nc.gpsimd.dma_start(bih_sb[:], bih[:].rearrange("g p -> p g"))
            bhh_sb = cst.tile([128, 3], f32)
            nc.gpsimd.dma_start(bhh_sb[:], bhh[:].rearrange("g p -> p g"))
            gib = cst.tile([128, 3], f32)
            nc.vector.tensor_tensor(out=gib[:], in0=gicol[:], in1=bih_sb[:], op=ALU.add)
            ghb = cst.tile([128, 3], f32)
            nc.vector.tensor_tensor(out=ghb[:], in0=ghcol[:], in1=bhh_sb[:], op=ALU.add)

            rzpre = cst.tile([128, 2], f32)
            nc.vector.tensor_tensor(out=rzpre[:], in0=gib[:, 0:2], in1=ghb[:, 0:2],
                                    op=ALU.add)
            rzg = cst.tile([128, 2], f32)
            nc.scalar.activation(rzg[:], rzpre[:], AF.Sigmoid)
            npre = cst.tile([128, 1], f32)
            nc.vector.tensor_tensor(out=npre[:], in0=rzg[:, 0:1], in1=ghb[:, 2:3],
                                    op=ALU.mult)
            nc.vector.tensor_tensor(out=npre[:], in0=npre[:], in1=gib[:, 2:3],
                                    op=ALU.add)
            ngate = cst.tile([128, 1], f32)
            nc.scalar.activation(ngate[:], npre[:], AF.Tanh)
            hsh_sb = cst.tile([128, 1], f32)
            nc.gpsimd.dma_start(hsh_sb[:], hsh[:])
            hmn = cst.tile([128, 1], f32)
            nc.vector.tensor_tensor(out=hmn[:], in0=hsh_sb[:], in1=ngate[:],
                                    op=ALU.subtract)
            zh = cst.tile([128, 1], f32)
            nc.vector.tensor_tensor(out=zh[:], in0=rzg[:, 1:2], in1=hmn[:],
                                    op=ALU.mult)
            hnew = cst.tile([128, 1], f32)
            nc.vector.tensor_tensor(out=hnew[:], in0=ngate[:], in1=zh[:], op=ALU.add)
            nc.sync.dma_start(o_hnew[:], hnew[:])

            # ---- 7. AllGather(h_new) ----
            c4_in = dram.tile([1, 128], f32)
            nc.gpsimd.dma_start(c4_in[0:1, :].rearrange("a b -> b a"), hnew[:])
            c4_out = dram.tile([1, H], f32, addr_space="Shared")
            nc.gpsimd.collective_compute(
                "AllGather", mybir.AluOpType.bypass, replica_groups=RG,
                ins=[c4_in[:].opt()], outs=[c4_out[:].opt()])

            # ---- 8. logits = [h_new; ctx] @ out_W_shard.T (PE, f32r) ----
            x2c = cst.tile([128, 2 * HC], f32r)
            nc.gpsimd.dma_start(x2c[:, 0:HC],
                                c4_out[0:1, :].rearrange("a (c p) -> (a p) c", p=128)
                                .bitcast(f32r))
            nc.gpsimd.dma_start(x2c[:, HC:2 * HC],
                                ctx_dram[0:1, :].rearrange("a (c p) -> (a p) c", p=128)
                                .bitcast(f32r))

            lg_row = row_p.tile([1, VS], f32, tag="lg", name="lg_row")
            nck = (VS + 1023) // 1024
            mcs = cst.tile([1, nck], f32)
            nmcs = cst.tile([1, nck], f32)
            scs = cst.tile([1, nck], f32)
            VBP = [(0, 2048), (2048, 2048), (4096, 2048), (6144, 256)]
            halves = [("c", list(range(HC, 2 * HC))), ("h", list(range(HC)))]
            for half, ks in halves:
                for off, wid in VBP:
                    nvb = (wid + 1023) // 1024
                    pss = [ps_acc.tile([1, 1024], f32, tag="acc",
                                       name=f"psl_{half}_{off}_{vb}", space="PSUM")
                           for vb in range(nvb)]
                    for i, k in enumerate(ks):
                        wo_fp = big_p.tile([128, 2048], f32r, tag="big",
                                           name=f"wofp_{half}_{off}_{k}")
                        dma(wo_fp[:, 0:wid],
                            wot[k * 128:(k + 1) * 128, off:off + wid].bitcast(f32r))
                        for vb in range(nvb):
                            w0 = vb * 1024
                            vw = min(1024, wid - w0)
                            for j in range(0, vw, 512):
                                nj = min(512, vw - j)
                                nc.tensor.matmul(
                                    pss[vb][0:1, j:j + nj],
                                    lhsT=x2c[:, k:k + 1],
                                    rhs=wo_fp[:, w0 + j:w0 + j + nj],
                                    start=(i == 0), stop=(i == HC - 1),
                                    skip_group_check=True)
                    for vb in range(nvb):
                        w0 = vb * 1024
                        vw = min(1024, wid - w0)
                        seg = lg_row[0:1, off + w0:off + w0 + vw]
                        if half == "c":
                            nc.vector.tensor_copy(seg, pss[vb][0:1, 0:vw])
                        else:
                            ci = (off + w0) // 1024
                            nc.vector.tensor_tensor(out=seg, in0=seg,
                                                    in1=pss[vb][0:1, 0:vw],
                                                    op=ALU.add)
                            ob = rowc_p.tile([1, 1024], f32, tag="obc",
                                             name=f"ob_{off}_{vb}")
                            nc.sync.dma_start(ob[0:1, 0:vw],
                                              outb[0:1, off + w0:off + w0 + vw])
                            nc.vector.tensor_tensor(out=seg, in0=seg,
                                                    in1=ob[0:1, 0:vw], op=ALU.add)
                            nc.vector.tensor_reduce(mcs[:, ci:ci + 1], seg,
                                                    axis=AX.X, op=ALU.max)
                            nc.vector.tensor_scalar_mul(nmcs[:, ci:ci + 1],
                                                        mcs[:, ci:ci + 1], -1.0)
                            ex = rowc_p.tile([1, 1024], f32, tag="obc",
                                             name=f"ex_{off}_{vb}")
                            nc.scalar.activation(ex[0:1, 0:vw], seg, AF.Exp,
                                                 bias=nmcs[:, ci:ci + 1], scale=1.0,
                                                 accum_out=scs[:, ci:ci + 1])
            if DEBUG:
                nc.sync.dma_start(d_lg[:], lg_row[:])

            # ---- 9. merge chunk stats + AllGather ----
            lm = cst.tile([1, 1], f32)
            nc.vector.tensor_reduce(lm[:], mcs[:], axis=AX.X, op=ALU.max)
            nlm = cst.tile([1, 1], f32)
            nc.vector.tensor_scalar_mul(nlm[:], lm[:], -1.0)
            dch = cst.tile([1, nck], f32)
            nc.scalar.activation(dch[:], mcs[:], AF.Exp, bias=nlm[:], scale=1.0)
            tch = cst.tile([1, nck], f32)
            nc.vector.tensor_tensor(out=tch[:], in0=dch[:], in1=scs[:], op=ALU.mult)
            ls = cst.tile([1, 1], f32)
            nc.vector.tensor_reduce(ls[:], tch[:], axis=AX.X, op=ALU.add)
            st_sb = cst.tile([1, 2], f32)
            nc.vector.tensor_copy(st_sb[:, 0:1], lm[:])
            nc.vector.tensor_copy(st_sb[:, 1:2], ls[:])
            st_in = dram.tile([1, 64], f32)
            nc.gpsimd.dma_start(st_in[0:1, 0:2], st_sb[:])
            st_out = dram.tile([1, 64 * NC], f32, addr_space="Shared")
            nc.gpsimd.collective_compute(
                "AllGather", mybir.AluOpType.bypass, replica_groups=RG,
                ins=[st_in[:].opt()], outs=[st_out[:].opt()])

            # ---- 10. global lse, final log-probs ----
            st_a = cst.tile([1, 64 * NC], f32)
            nc.gpsimd.dma_start(st_a[:], st_out[:])
            st_v = st_a[:].rearrange("a (j r) -> a r j", r=64)   # [1, 64, 8]
            mvals = cst.tile([1, NC], f32)
            nc.vector.tensor_copy(mvals[:], st_v[:, 0:1, :])
            svals = cst.tile([1, NC], f32)
            nc.vector.tensor_copy(svals[:], st_v[:, 1:2, :])
            gM = cst.tile([1, 1], f32)
            nc.vector.tensor_reduce(gM[:], mvals[:], axis=AX.X, op=ALU.max)
            ngM = cst.tile([1, 1], f32)
            nc.vector.tensor_scalar_mul(ngM[:], gM[:], -1.0)
            dvals = cst.tile([1, NC], f32)
            nc.scalar.activation(dvals[:], mvals[:], AF.Exp, bias=ngM[:], scale=1.0)
            tvals = cst.tile([1, NC], f32)
            nc.vector.tensor_tensor(out=tvals[:], in0=dvals[:], in1=svals[:],
                                    op=ALU.mult)
            gZ = cst.tile([1, 1], f32)
            nc.vector.tensor_reduce(gZ[:], tvals[:], axis=AX.X, op=ALU.add)
            lnZ = cst.tile([1, 1], f32)
            nc.scalar.activation(lnZ[:], gZ[:], AF.Ln)
            lse = cst.tile([1, 1], f32)
            nc.vector.tensor_tensor(out=lse[:], in0=lnZ[:], in1=gM[:], op=ALU.add)
            nlse = cst.tile([1, 1], f32)
            nc.vector.tensor_scalar_mul(nlse[:], lse[:], -1.0)
            nc.scalar.activation(lg_row[:], lg_row[:], AF.Identity,
                                 bias=nlse[:], scale=1.0)
            nc.sync.dma_start(o_logp[:], lg_row[:])

    nc.compile()
    return nc


def _prep_in_maps(inputs):
    wi = np.asarray(inputs["word_input"]).astype(np.int64).reshape(-1)
    emb = np.asarray(inputs["emb"], dtype=np.float32)
    enc = np.asarray(inputs["encoder_outputs"], dtype=np.float32).reshape(L, H)
    attn_W = np.asarray(inputs["attn_W"], dtype=np.float32)
    v = np.asarray(inputs["v"], dtype=np.float32).reshape(1, H)
    W_ih = np.asarray(inputs["W_ih"], dtype=np.float32)
    W_hh = np.asarray(inputs["W_hh"], dtype=np.float32)
    b_ih = np.asarray(inputs["b_ih"], dtype=np.float32).reshape(-1)
    b_hh = np.asarray(inputs["b_hh"], dtype=np.float32).reshape(-1)
    h = np.asarray(inputs["last_hidden"], dtype=np.float32).reshape(1, H)
    out_W = np.asarray(inputs["out_W"], dtype=np.float32)
    out_b = np.asarray(inputs["out_b"], dtype=np.float32).reshape(-1)

    idx2 = np.full((2, 1), int(wi[0]), dtype=np.int32)
    w2T = np.ascontiguousarray(attn_W[:, H:2 * H].T)
    encT = np.ascontiguousarray(enc.T)

    Wpad = np.zeros((VP, 2 * H), dtype=np.float32)
    Wpad[:V] = out_W
    bpad = np.full((VP,), -1e30, dtype=np.float32)
    bpad[:V] = out_b

    in_maps = []
    for k in range(NC):
        rows = np.concatenate([np.arange(g * H + k * 128, g * H + (k + 1) * 128)
                               for g in range(3)])
        in_maps.append({
            "idx2": idx2,
            "emb_cs": np.ascontiguousarray(emb[:, k * 128:(k + 1) * 128]),
            "encT": encT,
            "w2T": w2T,
            "vvec": v,
            "wih": np.ascontiguousarray(W_ih[rows]),
            "whh": np.ascontiguousarray(W_hh[rows]),
            "bih": np.ascontiguousarray(b_ih[rows].reshape(3, 128)),
            "bhh": np.ascontiguousarray(b_hh[rows].reshape(3, 128)),
            "hfull": h,
            "hsh": np.ascontiguousarray(h[0, k * 128:(k + 1) * 128].reshape(128, 1)),
            "wot": np.ascontiguousarray(Wpad[k * VS:(k + 1) * VS].T),
            "outb": np.ascontiguousarray(bpad[k * VS:(k + 1) * VS].reshape(1, VS)),
        })
    return in_maps


@contextlib.contextmanager
def _maybe_profile():
    prof_dir = os.environ.get("NN_PROF_DIR")
    if not prof_dir:
        yield
        return
    import jax
    jax.devices()
    lib = ctypes.CDLL("/opt/axon/libaxon_pjrt.so")
    lib.axon_start_nrt_profile.argtypes = [ctypes.POINTER(ctypes.c_int64),
                                           ctypes.c_size_t]
    lib.axon_start_nrt_profile.restype = ctypes.c_int64
    lib.axon_stop_nrt_profile.argtypes = [ctypes.c_char_p]
    lib.axon_stop_nrt_profile.restype = ctypes.c_int64
    ids = (ctypes.c_int64 * 1)(0)
    rc = lib.axon_start_nrt_profile(ids, 1)
    if rc != 0:
        raise RuntimeError(f"axon_start_nrt_profile rc={rc}")
    try:
        yield
    finally:
        n = lib.axon_stop_nrt_profile(str(prof_dir).encode())
        print(f"profile: {n} file(s) written to {prof_dir}")


def kernel(**inputs):
    from concourse import bass_utils

    if "nc" not in _CACHE:
        _CACHE["nc"] = _build()
    nc = _CACHE["nc"]
    in_maps = _prep_in_maps(inputs)
    with _maybe_profile():
        res = bass_utils.run_bass_kernel_spmd(nc, in_maps, core_ids=list(range(NC)))

    if DEBUG:
        _CACHE["last_results"] = res.results

    logp = np.concatenate([res.results[k]["o_logp"].reshape(VS) for k in range(NC)])
    log_probs = logp[:V].reshape(1, V)
    h_new = np.concatenate([res.results[k]["o_hnew"].reshape(128)
                            for k in range(NC)]).reshape(1, 1, H)
    attn_w = res.results[0]["o_attnw"].reshape(L).reshape(1, 1, L)
    return log_probs, h_new, attn_w


# revision 17
# speedup vs baseline: 1.1120x; 1.0782x over previous
"""Trainium2 Bass kernel for a Bahdanau attention decoder step (B=1).

Shapes (hardcoded): H=1024, V=50257, L=2048, B=1, 8 NeuronCores.

Sharding:
  - vocab dim padded to 51200 and sharded 6400/core for the output
    projection (out_W staged pre-transposed per core: [2H, 6400]).
  - embedding column-sharded [V, 128] per core; the looked-up row slice is
    AllGather'ed into the full embedded vector `we`.
  - GRU sharded by gate-slice: core k computes rows k*128:(k+1)*128 of each
    of the r/z/n gates and its h_new slice; h_new is AllGather'ed.
  - attention replicated (encoder_outputs staged transposed + attn_W[:, H:]).

Collectives: AllGather(we) at t~0 (overlapped), AllGather(h_new),
AllGather(logsumexp stats). log_softmax = logits - (M + log sum_j S_j e^{m_j-M}).

Perf notes: DMA spread across the sync+scalar HWDGE queues in ~1MB tiles; the
vocab projection and attention-score matmuls stream as float32r (same fp32
bytes, PE streams 1 row/cycle vs 4 for strict fp32); context and GRU math stay
strict fp32; softmax and log-softmax stats run on partition 0 (no transposes).
"""
import sys, os, ctypes, contextlib

if "/opt/trn_rl_repo" not in sys.path:
    sys.path.insert(0, "/opt/trn_rl_repo")

import numpy as np

H = 1024
V = 50257
L = 2048
NC = 8
VP = 51200          # padded vocab
VS = VP // NC       # 6400 per core
HC = H // 128       # 8 hidden chunks
DEBUG = bool(int(os.environ.get("NN_KERNEL_DEBUG", "0")))

_CACHE = {}


def _build():
    import concourse.bass as bass
    import concourse.bacc as bacc
    import concourse.mybir as mybir
    import concourse.tile as tile

    f32 = mybir.dt.float32
    f32r = mybir.dt.float32r
    i32 = mybir.dt.int32
    AF = mybir.ActivationFunctionType
    ALU = mybir.AluOpType
    AX = mybir.AxisListType

    nc = bacc.Bacc("TRN2", target_bir_lowering=False, debug=False, num_devices=NC)

    # ---- inputs (per-core shards staged by host) ----
    idx2 = nc.dram_tensor("idx2", [2, 1], i32, kind="ExternalInput")
    emb_cs = nc.dram_tensor("emb_cs", [V, 128], f32, kind="ExternalInput")
    encT = nc.dram_tensor("encT", [H, L], f32, kind="ExternalInput")
    w2T = nc.dram_tensor("w2T", [H, H], f32, kind="ExternalInput")
    vvec = nc.dram_tensor("vvec", [1, H], f32, kind="ExternalInput")
    wih = nc.dram_tensor("wih", [384, 2 * H], f32, kind="ExternalInput")
    whh = nc.dram_tensor("whh", [384, H], f32, kind="ExternalInput")
    bih = nc.dram_tensor("bih", [3, 128], f32, kind="ExternalInput")
    bhh = nc.dram_tensor("bhh", [3, 128], f32, kind="ExternalInput")
    hfull = nc.dram_tensor("hfull", [1, H], f32, kind="ExternalInput")
    hsh = nc.dram_tensor("hsh", [128, 1], f32, kind="ExternalInput")
    wot = nc.dram_tensor("wot", [2 * H, VS], f32, kind="ExternalInput")
    outb = nc.dram_tensor("outb", [1, VS], f32, kind="ExternalInput")

    # ---- outputs ----
    o_logp = nc.dram_tensor("o_logp", [1, VS], f32, kind="ExternalOutput")
    o_hnew = nc.dram_tensor("o_hnew", [128, 1], f32, kind="ExternalOutput")
    o_attnw = nc.dram_tensor("o_attnw", [1, L], f32, kind="ExternalOutput")
    if DEBUG:
        d_scores = nc.dram_tensor("d_scores", [1, L], f32, kind="ExternalOutput")
        d_ctx = nc.dram_tensor("d_ctx", [1, H], f32, kind="ExternalOutput")
        d_lg = nc.dram_tensor("d_lg", [1, VS], f32, kind="ExternalOutput")

    RG = [list(range(NC))]

    with tile.TileContext(nc) as tc:
        with tc.tile_pool(name="dram", bufs=1, space="DRAM") as dram, \
             tc.tile_pool(name="cst", bufs=1) as cst, \
             tc.tile_pool(name="big_p", bufs=14) as big_p, \
             tc.tile_pool(name="scr", bufs=1) as scr_p, \
             tc.tile_pool(name="bc_p", bufs=2) as bc_p, \
             tc.tile_pool(name="row_p", bufs=1) as row_p, \
             tc.tile_pool(name="rowc_p", bufs=2) as rowc_p, \
             tc.tile_pool(name="ps_acc", bufs=4, space="PSUM") as ps_acc:

            dmai = [0]
            dma_engs = [nc.sync, nc.scalar]

            def dma(out_ap, in_ap):
                e = dma_engs[dmai[0] % 2]
                dmai[0] += 1
                e.dma_start(out_ap, in_ap)

            # ---- 1. embedding gather + AllGather(we) (fires immediately) ----
            idx_sb = cst.tile([2, 1], i32)
            nc.gpsimd.dma_start(idx_sb[:], idx2[:])
            we_g = cst.tile([2, 128], f32)
            nc.gpsimd.indirect_dma_start(
                out=we_g[:], out_offset=None, in_=emb_cs[:],
                in_offset=bass.IndirectOffsetOnAxis(ap=idx_sb[:, :1], axis=0))
            cwe_in = dram.tile([1, 128], f32)
            cwe_out = dram.tile([1, H], f32, addr_space="Shared")
            nc.gpsimd.dma_start(cwe_in[:], we_g[0:1, :])
            nc.gpsimd.collective_compute(
                "AllGather", mybir.AluOpType.bypass, replica_groups=RG,
                ins=[cwe_in[:].opt()], outs=[cwe_out[:].opt()])

            # ---- 2. u2 = v @ W2 (DVE over W2T tiles, direct chunk layout) ----
            v_rep = bc_p.tile([128, 2 * H], f32, tag="bc", name="v_rep")[:, 0:H]
            nc.gpsimd.dma_start(v_rep[:], vvec[0:1, :].to_broadcast([128, H]))
            u2c_f = cst.tile([128, HC], f32)
            for mp in range(4):
                w2_t = big_p.tile([128, 2 * H], f32, tag="big", name=f"w2_{mp}")
                dma(w2_t[:].rearrange("p (b h) -> p b h", b=2),
                    w2T[mp * 256:(mp + 1) * 256, :]
                    .rearrange("(b p) h -> p b h", p=128))
                for b in range(2):
                    mc = 2 * mp + b
                    sc = scr_p.tile([128, 2 * H], f32, tag="scr", name=f"uscr_{mc}")
                    nc.vector.tensor_tensor(out=sc[:, 0:H],
                                            in0=w2_t[:, b * H:(b + 1) * H],
                                            in1=v_rep[:], op=ALU.mult)
                    nc.vector.tensor_reduce(u2c_f[:, mc:mc + 1], sc[:, 0:H],
                                            axis=AX.X, op=ALU.add)
            u2_c = cst.tile([128, HC], f32r)
            nc.scalar.copy(u2_c[:], u2c_f[:])

            # ---- 3. scores = enc @ u2 (PE, f32r over resident encT tiles) ----
            enc_t = []
            ps_s = [ps_acc.tile([1, 1024], f32, tag="acc", name=f"ps_s{i}",
                                space="PSUM") for i in range(2)]
            for hc in range(HC):
                et = big_p.tile([128, L], f32r, tag="big", name=f"enc_{hc}")
                dma(et[:], encT[hc * 128:(hc + 1) * 128, :].bitcast(f32r))
                enc_t.append(et)
                for j in range(0, L, 512):
                    nc.tensor.matmul(ps_s[j // 1024][0:1, j % 1024:j % 1024 + 512],
                                     lhsT=u2_c[:, hc:hc + 1],
                                     rhs=et[:, j:j + 512],
                                     start=(hc == 0), stop=(hc == HC - 1),
                                     skip_group_check=True)
            scores = cst.tile([1, L], f32)
            nc.vector.tensor_copy(scores[:, 0:1024], ps_s[0][0:1, :])
            nc.vector.tensor_copy(scores[:, 1024:2048], ps_s[1][0:1, :])
            if DEBUG:
                nc.sync.dma_start(d_scores[:], scores[:])

            # ---- 4. softmax(scores) on partition 0 ----
            sm = cst.tile([1, 1], f32)
            nc.vector.tensor_reduce(sm[:], scores[:], axis=AX.X, op=ALU.max)
            nsm = cst.tile([1, 1], f32)
            nc.vector.tensor_scalar_mul(nsm[:], sm[:], -1.0)
            exps = cst.tile([1, L], f32)
            zs = cst.tile([1, 1], f32)
            nc.scalar.activation(exps[:], scores[:], AF.Exp,
                                 bias=nsm[:], scale=1.0, accum_out=zs[:])
            exps_dram = dram.tile([1, L], f32)
            nc.gpsimd.dma_start(exps_dram[:], exps[:])
            exps_rep = bc_p.tile([128, 2 * H], f32, tag="bc", name="exps_rep")[:, 0:L]
            nc.gpsimd.dma_start(exps_rep[:], exps_dram[:].to_broadcast([128, L]))
            rz = cst.tile([1, 1], f32)
            nc.vector.reciprocal(rz[:], zs[:])
            attw_row = cst.tile([1, L], f32)
            nc.scalar.mul(attw_row[:], exps[:], rz[:])
            nc.sync.dma_start(o_attnw[:], attw_row[:])
            rz_dram = dram.tile([1, 1], f32)
            nc.gpsimd.dma_start(rz_dram[:], rz[:])
            rz_bc = cst.tile([128, 1], f32)
            nc.gpsimd.dma_start(rz_bc[:], rz_dram[:].to_broadcast([128, 1]))

            # ---- 5. context = attw @ enc (DVE, fp32 exact) ----
            ctx_u = cst.tile([128, HC], f32)
            for hc in range(HC):
                sc = scr_p.tile([128, 2 * H], f32, tag="scr", name=f"cscr_{hc}")
                nc.vector.tensor_tensor(out=sc[:], in0=enc_t[hc][:].bitcast(f32),
                                        in1=exps_rep[:], op=ALU.mult)
                nc.vector.tensor_reduce(ctx_u[:, hc:hc + 1], sc[:], axis=AX.X,
                                        op=ALU.add)
            ctx_c = cst.tile([128, HC], f32)
            nc.vector.tensor_scalar(out=ctx_c[:], in0=ctx_u[:], scalar1=rz_bc[:, 0:1],
                                    scalar2=None, op0=ALU.mult)
            ctx_dram = dram.tile([1, H], f32)
            nc.gpsimd.dma_start(ctx_dram[0:1, :].rearrange("a (c p) -> (a p) c", p=128),
                                ctx_c[:])
            if DEBUG:
                nc.sync.dma_start(d_ctx[:], ctx_dram[:])

            # ---- 6. GRU slice-k (DVE, fp32) ----
            h_rep = bc_p.tile([128, 2 * H], f32, tag="bc", name="h_rep")[:, 0:H]
            nc.gpsimd.dma_start(h_rep[:], hfull[0:1, :].to_broadcast([128, H]))
            ghcol = cst.tile([128, 3], f32)
            wt01 = big_p.tile([128, 2 * H], f32, tag="big", name="whh01")
            dma(wt01[:].rearrange("p (b h) -> p b h", b=2),
                whh[0:256, :].rearrange("(b p) h -> p b h", p=128))
            for g in range(2):
                sc = scr_p.tile([128, 2 * H], f32, tag="scr", name=f"ghscr_{g}")
                nc.vector.tensor_tensor(out=sc[:, 0:H], in0=wt01[:, g * H:(g + 1) * H],
                                        in1=h_rep[:], op=ALU.mult)
                nc.vector.tensor_reduce(ghcol[:, g:g + 1], sc[:, 0:H], axis=AX.X,
                                        op=ALU.add)
            wt2 = big_p.tile([128, 2 * H], f32, tag="big", name="whh2")
            dma(wt2[:, 0:H], whh[256:384, :])
            sc = scr_p.tile([128, 2 * H], f32, tag="scr", name="ghscr_2")
            nc.vector.tensor_tensor(out=sc[:, 0:H], in0=wt2[:, 0:H], in1=h_rep[:],
                                    op=ALU.mult)
            nc.vector.tensor_reduce(ghcol[:, 2:3], sc[:, 0:H], axis=AX.X, op=ALU.add)

            x_rep = bc_p.tile([128, 2 * H], f32, tag="bc", name="x_rep")
            nc.gpsimd.dma_start(x_rep[:, 0:H], cwe_out[:].to_broadcast([128, H]))
            nc.gpsimd.dma_start(x_rep[:, H:2 * H], ctx_dram[:].to_broadcast([128, H]))
            gicol = cst.tile([128, 3], f32)
            for g in range(3):
                wt = big_p.tile([128, 2 * H], f32, tag="big", name=f"wih_{g}")
                dma(wt[:], wih[g * 128:(g + 1) * 128, :])
                sc = scr_p.tile([128, 2 * H], f32, tag="scr", name=f"giscr_{g}")
                nc.vector.tensor_tensor(out=sc[:], in0=wt[:], in1=x_rep[:],
                                        op=ALU.mult)
                nc.vector.tensor_reduce(gicol[:, g:g + 1], sc[:], axis=AX.X,
                                        op=ALU.add)

            bih_sb = cst.tile([128, 3], f32)
            nc.gpsimd.dma_start(bih_sb[:], bih[:].rearrange("g p -> p g"))
            bhh_sb = cst.tile([128, 3], f32)
            nc.gpsimd.dma_start(bhh_sb[:], bhh[:].rearrange("g p -> p g"))
            gib = cst.tile([128, 3], f32)
            nc.vector.tensor_tensor(out=gib[:], in0=gicol[:], in1=bih_sb[:], op=ALU.add)
            ghb = cst.tile([128, 3], f32)
            nc.vector.tensor_tensor(out=ghb[:], in0=ghcol[:], in1=bhh_sb[:], op=ALU.add)

            rzpre = cst.tile([128, 2], f32)
            nc.vector.tensor_tensor(out=rzpre[:], in0=gib[:, 0:2], in1=ghb[:, 0:2],
                                    op=ALU.add)
            rzg = cst.tile([128, 2], f32)
            nc.scalar.activation(rzg[:], rzpre[:], AF.Sigmoid)
            npre = cst.tile([128, 1], f32)
            nc.vector.tensor_tensor(out=npre[:], in0=rzg[:, 0:1], in1=ghb[:, 2:3],
                                    op=ALU.mult)
            nc.vector.tensor_tensor(out=npre[:], in0=npre[:], in1=gib[:, 2:3],
                                    op=ALU.add)
            ngate = cst.tile([128, 1], f32)
            nc.scalar.activation(ngate[:], npre[:], AF.Tanh)
            hsh_sb = cst.tile([128, 1], f32)
            nc.gpsimd.dma_start(hsh_sb[:], hsh[:])
            hmn = cst.tile([128, 1], f32)
            nc.vector.tensor_tensor(out=hmn[:], in0=hsh_sb[:], in1=ngate[:],
                                    op=ALU.subtract)
            zh = cst.tile([128, 1], f32)
            nc.vector.tensor_tensor(out=zh[:], in0=rzg[:, 1:2], in1=hmn[:],
                                    op=ALU.mult)
            hnew = cst.tile([128, 1], f32)
            nc.vector.tensor_tensor(out=hnew[:], in0=ngate[:], in1=zh[:], op=ALU.add)
            nc.sync.dma_start(o_hnew[:], hnew[:])

            # ---- 7. AllGather(h_new) ----
            c4_in = dram.tile([1, 128], f32)
            nc.gpsimd.dma_start(c4_in[0:1, :].rearrange("a b -> b a"), hnew[:])
            c4_out = dram.tile([1, H], f32, addr_space="Shared")
            nc.gpsimd.collective_compute(
                "AllGather", mybir.AluOpType.bypass, replica_groups=RG,
                ins=[c4_in[:].opt()], outs=[c4_out[:].opt()])

            # ---- 8. logits = [h_new; ctx] @ out_W_shard.T (PE, f32r) ----
            x2c = cst.tile([128, 2 * HC], f32r)
            nc.gpsimd.dma_start(x2c[:, 0:HC],
                                c4_out[0:1, :].rearrange("a (c p) -> (a p) c", p=128)
                                .bitcast(f32r))
            nc.gpsimd.dma_start(x2c[:, HC:2 * HC],
                                ctx_dram[0:1, :].rearrange("a (c p) -> (a p) c", p=128)
                                .bitcast(f32r))

            lg_row = row_p.tile([1, VS], f32, tag="lg", name="lg_row")
            nck = (VS + 1023) // 1024
            mcs = cst.tile([1, nck], f32)
            nmcs = cst.tile([1, nck], f32)
            scs = cst.tile([1, nck], f32)
            VBP = [(0, 2048), (2048, 2048), (4096, 2048), (6144, 256)]
            halves = [("c", list(range(HC, 2 * HC))), ("h", list(range(HC)))]
            for half, ks in halves:
                for off, wid in VBP:
                    nvb = (wid + 1023) // 1024
                    pss = [ps_acc.tile([1, 1024], f32, tag="acc",
                                       name=f"psl_{half}_{off}_{vb}", space="PSUM")
                           for vb in range(nvb)]
                    for i, k in enumerate(ks):
                        wo_fp = big_p.tile([128, 2048], f32r, tag="big",
                                           name=f"wofp_{half}_{off}_{k}")
                        dma(wo_fp[:, 0:wid],
                            wot[k * 128:(k + 1) * 128, off:off + wid].bitcast(f32r))
                        for vb in range(nvb):
                            w0 = vb * 1024
                            vw = min(1024, wid - w0)
                            for j in range(0, vw, 512):
                                nj = min(512, vw - j)
                                nc.tensor.matmul(
                                    pss[vb][0:1, j:j + nj],
                                    lhsT=x2c[:, k:k + 1],
                                    rhs=wo_fp[:, w0 + j:w0 + j + nj],
                                    start=(i == 0), stop=(i == HC - 1),
                                    skip_group_check=True)
                    for vb in range(nvb):
                        w0 = vb * 1024
                        vw = min(1024, wid - w0)
                        seg = lg_row[0:1, off + w0:off + w0 + vw]
                        if half == "c":
                            nc.vector.tensor_copy(seg, pss[vb][0:1, 0:vw])
                        else:
                            ci = (off + w0) // 1024
                            nc.vector.tensor_tensor(out=seg, in0=seg,
                                                    in1=pss[vb][0:1, 0:vw],
                                                    op=ALU.add)
                            ob = rowc_p.tile([1, 1024], f32, tag="obc",
                                             name=f"ob_{off}_{vb}")
                            nc.sync.dma_start(ob[0:1, 0:vw],
                                              outb[0:1, off + w0:off + w0 + vw])
                            nc.vector.tensor_tensor(out=seg, in0=seg,
                                                    in1=ob[0:1, 0:vw], op=ALU.add)
                            nc.vector.tensor_reduce(mcs[:, ci:ci + 1], seg,
                                                    axis=AX.X, op=ALU.max)
                            nc.vector.tensor_scalar_mul(nmcs[:, ci:ci + 1],
                                                        mcs[:, ci:ci + 1], -1.0)
                            ex = rowc_p.tile([1, 1024], f32, tag="obc",
                                             name=f"ex_{off}_{vb}")
                            nc.scalar.activation(ex[0:1, 0:vw], seg, AF.Exp,
                                                 bias=nmcs[:, ci:ci + 1], scale=1.0,
                                                 accum_out=scs[:, ci:ci + 1])
            if DEBUG:
                nc.sync.dma_start(d_lg[:], lg_row[:])

            # ---- 9. merge chunk stats + AllGather ----
            lm = cst.tile([1, 1], f32)
            nc.vector.tensor_reduce(lm[:], mcs[:], axis=AX.X, op=ALU.max)
            nlm = cst.tile([1, 1], f32)
            nc.vector.tensor_scalar_mul(nlm[:], lm[:], -1.0)
            dch = cst.tile([1, nck], f32)
            nc.scalar.activation(dch[:], mcs[:], AF.Exp, bias=nlm[:], scale=1.0)
            tch = cst.tile([1, nck], f32)
            nc.vector.tensor_tensor(out=tch[:], in0=dch[:], in1=scs[:], op=ALU.mult)
            ls = cst.tile([1, 1], f32)
            nc.vector.tensor_reduce(ls[:], tch[:], axis=AX.X, op=ALU.add)
            st_sb = cst.tile([1, 2], f32)
            nc.vector.tensor_copy(st_sb[:, 0:1], lm[:])
            nc.vector.tensor_copy(st_sb[:, 1:2], ls[:])
            st_in = dram.tile([1, 64], f32)
            nc.gpsimd.dma_start(st_in[0:1, 0:2], st_sb[:])
            st_out = dram.tile([1, 64 * NC], f32, addr_space="Shared")
            nc.gpsimd.collective_compute(
                "AllGather", mybir.AluOpType.bypass, replica_groups=RG,
                ins=[st_in[:].opt()], outs=[st_out[:].opt()])

            # ---- 10. global lse, final log-probs ----
            st_a = cst.tile([1, 64 * NC], f32)
            nc.gpsimd.dma_start(st_a[:], st_out[:])
            st_v = st_a[:].rearrange("a (j r) -> a r j", r=64)   # [1, 64, 8]
            mvals = cst.tile([1, NC], f32)
            nc.vector.tensor_copy(mvals[:], st_v[:, 0:1, :])
            svals = cst.tile([1, NC], f32)
            nc.vector.tensor_copy(svals[:], st_v[:, 1:2, :])
            gM = cst.tile([1, 1], f32)
            nc.vector.tensor_reduce(gM[:], mvals[:], axis=AX.X, op=ALU.max)
            ngM = cst.tile([1, 1], f32)
            nc.vector.tensor_scalar_mul(ngM[:], gM[:], -1.0)
            dvals = cst.tile([1, NC], f32)
            nc.scalar.activation(dvals[:], mvals[:], AF.Exp, bias=ngM[:], scale=1.0)
            tvals = cst.tile([1, NC], f32)
            nc.vector.tensor_tensor(out=tvals[:], in0=dvals[:], in1=svals[:],
                                    op=ALU.mult)
            gZ = cst.tile([1, 1], f32)
            nc.vector.tensor_reduce(gZ[:], tvals[:], axis=AX.X, op=ALU.add)
            lnZ = cst.tile([1, 1], f32)
            nc.scalar.activation(lnZ[:], gZ[:], AF.Ln)
            lse = cst.tile([1, 1], f32)
            nc.vector.tensor_tensor(out=lse[:], in0=lnZ[:], in1=gM[:], op=ALU.add)
            nlse = cst.tile([1, 1], f32)
            nc.vector.tensor_scalar_mul(nlse[:], lse[:], -1.0)
            nc.scalar.activation(lg_row[:], lg_row[:], AF.Identity,
                                 bias=nlse[:], scale=1.0)
            nc.sync.dma_start(o_logp[:], lg_row[:])

    nc.compile()
    return nc


def _prep_in_maps(inputs):
    wi = np.asarray(inputs["word_input"]).astype(np.int64).reshape(-1)
    emb = np.asarray(inputs["emb"], dtype=np.float32)
    enc = np.asarray(inputs["encoder_outputs"], dtype=np.float32).reshape(L, H)
    attn_W = np.asarray(inputs["attn_W"], dtype=np.float32)
    v = np.asarray(inputs["v"], dtype=np.float32).reshape(1, H)
    W_ih = np.asarray(inputs["W_ih"], dtype=np.float32)
    W_hh = np.asarray(inputs["W_hh"], dtype=np.float32)
    b_ih = np.asarray(inputs["b_ih"], dtype=np.float32).reshape(-1)
    b_hh = np.asarray(inputs["b_hh"], dtype=np.float32).reshape(-1)
    h = np.asarray(inputs["last_hidden"], dtype=np.float32).reshape(1, H)
    out_W = np.asarray(inputs["out_W"], dtype=np.float32)
    out_b = np.asarray(inputs["out_b"], dtype=np.float32).reshape(-1)

    idx2 = np.full((2, 1), int(wi[0]), dtype=np.int32)
    w2T = np.ascontiguousarray(attn_W[:, H:2 * H].T)
    encT = np.ascontiguousarray(enc.T)

    Wpad = np.zeros((VP, 2 * H), dtype=np.float32)
    Wpad[:V] = out_W
    bpad = np.full((VP,), -1e30, dtype=np.float32)
    bpad[:V] = out_b

    in_maps = []
    for k in range(NC):
        rows = np.concatenate([np.arange(g * H + k * 128, g * H + (k + 1) * 128)
                               for g in range(3)])
        in_maps.append({
            "idx2": idx2,
            "emb_cs": np.ascontiguousarray(emb[:, k * 128:(k + 1) * 128]),
            "encT": encT,
            "w2T": w2T,
            "vvec": v,
            "wih": np.ascontiguousarray(W_ih[rows]),
            "whh": np.ascontiguousarray(W_hh[rows]),
            "bih": np.ascontiguousarray(b_ih[rows].reshape(3, 128)),
            "bhh": np.ascontiguousarray(b_hh[rows].reshape(3, 128)),
            "hfull": h,
            "hsh": np.ascontiguousarray(h[0, k * 128:(k + 1) * 128].reshape(128, 1)),
            "wot": np.ascontiguousarray(Wpad[k * VS:(k + 1) * VS].T),
            "outb": np.ascontiguousarray(bpad[k * VS:(k + 1) * VS].reshape(1, VS)),
        })
    return in_maps


@contextlib.contextmanager
def _maybe_profile():
    prof_dir = os.environ.get("NN_PROF_DIR")
    if not prof_dir:
        yield
        return
    import jax
    jax.devices()
    lib = ctypes.CDLL("/opt/axon/libaxon_pjrt.so")
    lib.axon_start_nrt_profile.argtypes = [ctypes.POINTER(ctypes.c_int64),
                                           ctypes.c_size_t]
    lib.axon_start_nrt_profile.restype = ctypes.c_int64
    lib.axon_stop_nrt_profile.argtypes = [ctypes.c_char_p]
    lib.axon_stop_nrt_profile.restype = ctypes.c_int64
    ids = (ctypes.c_int64 * 1)(0)
    rc = lib.axon_start_nrt_profile(ids, 1)
    if rc != 0:
        raise RuntimeError(f"axon_start_nrt_profile rc={rc}")
    try:
        yield
    finally:
        n = lib.axon_stop_nrt_profile(str(prof_dir).encode())
        print(f"profile: {n} file(s) written to {prof_dir}")


def kernel(**inputs):
    from concourse import bass_utils

    if "nc" not in _CACHE:
        _CACHE["nc"] = _build()
    nc = _CACHE["nc"]
    in_maps = _prep_in_maps(inputs)
    with _maybe_profile():
        res = bass_utils.run_bass_kernel_spmd(nc, in_maps, core_ids=list(range(NC)))

    if DEBUG:
        _CACHE["last_results"] = res.results

    logp = np.concatenate([res.results[k]["o_logp"].reshape(VS) for k in range(NC)])
    log_probs = logp[:V].reshape(1, V)
    h_new = np.concatenate([res.results[k]["o_hnew"].reshape(128)
                            for k in range(NC)]).reshape(1, 1, H)
    attn_w = res.results[0]["o_attnw"].reshape(L).reshape(1, 1, L)
    return log_probs, h_new, attn_w
